# revision 10
# baseline (speedup 1.0000x reference)
"""Trainium2 Bass kernel for per-pixel MDN head (nn_MDN_38946763440904).

Reference computation (per pixel, channels-first):
  h      = relu(W1 @ x5 + b1)        # 5  -> 32
  h      = relu(W2 @ h + b2)         # 32 -> 32
  latent = relu(W3 @ h + b3)         # 32 -> 32
  for c in (r, g, b):
      mu_c    = Wmu_c @ latent + bmu_c + x[c]
      sigma_c = softplus(Wsg_c @ latent + bsg_c)
      pi_c    = softmax(Wpi_c @ latent + bpi_c)   # over the 16 components

Strategy: shard H across the 8 cores (each core gets [4, 5, 64, 512]).
On-core, pixels are processed in supertiles of 4 groups x 2048 pixels.
Each group of 32 latent channels occupies 32 SBUF partitions, so all
matmuls are dense 128-partition block-diagonal fp32r matmuls.
Head outputs are packed into 128-row chunks:
  A = [mu_r | mu_g]   E = [mu_b]      (written straight from PSUM,
                                       residual+bias folded into matmuls)
  B = [sg_r | sg_g]   C = [sg_b | pi_r]   D = [pi_g | pi_b]
softplus = Ln(Exp(z + b) + 1); softmax normalizer 1/s = Exp(-Ln(s));
all ACT functions live in one table set (natural_log_exp_and_others).
"""

import sys

if "/opt/trn_rl_repo" not in sys.path:
    sys.path.insert(0, "/opt/trn_rl_repo")

import numpy as np

import concourse.bass as bass
import concourse.mybir as mybir
import concourse.tile as tile
from concourse import bacc
from concourse.bass_utils import run_bass_kernel_spmd

F32 = mybir.dt.float32
F32R = mybir.dt.float32r
AF = mybir.ActivationFunctionType
ALU = mybir.AluOpType

B, CIN, H, W = 4, 5, 512, 512
K, LAT = 16, 32
NCORES = 8
HC = H // NCORES            # 64 rows of H per core
PXB = HC * W                # 32768 pixels per batch image per core
G = 4                       # pixel groups per supertile
COLS = 2048                 # pixels per group per supertile
NQ = COLS // 512            # 512-px quarters per group
ST_PER_B = PXB // (G * COLS)  # supertiles per batch image (4)

_CACHE = {}


def _build_program(trace=False):
    nc = bacc.Bacc("TRN2", target_bir_lowering=False, debug=False)

    xin = nc.dram_tensor("xin", [B, CIN + 1, PXB], F32R, kind="ExternalInput")

    wnames_r = {
        "lw1": [24, 128], "lw2": [128, 128], "lw3": [128, 128],
        "lA": [128, 128], "lB": [128, 128], "lC": [128, 128],
        "lD": [128, 128], "lE": [128, 64],
        "lrA": [24, 128], "lrE": [24, 64],
        "lsC": [64, 12], "lsD": [128, 12],
        "lbC": [12, 64], "lbD": [12, 128],
    }
    wnames_f = {
        "bb2": [128, 1], "bb3": [128, 1],
        "bB": [128, 1], "bC": [128, 1], "bD": [128, 1],
    }
    dram_w = {}
    for n, shp in wnames_r.items():
        dram_w[n] = nc.dram_tensor(n, shp, F32R, kind="ExternalInput")
    for n, shp in wnames_f.items():
        dram_w[n] = nc.dram_tensor(n, shp, F32, kind="ExternalInput")

    oA = nc.dram_tensor("oA", [2, B, K, PXB], F32, kind="ExternalOutput")
    oE = nc.dram_tensor("oE", [1, B, K, PXB], F32, kind="ExternalOutput")
    oB = nc.dram_tensor("oB", [2, B, K, PXB], F32, kind="ExternalOutput")
    oC = nc.dram_tensor("oC", [2, B, K, PXB], F32R, kind="ExternalOutput")
    oD = nc.dram_tensor("oD", [2, B, K, PXB], F32R, kind="ExternalOutput")

    from contextlib import ExitStack
    with tile.TileContext(nc) as tc, ExitStack() as es:
        consts = es.enter_context(tc.tile_pool(name="consts", bufs=1))
        xpool = es.enter_context(tc.tile_pool(name="xp", bufs=2))
        hpool = es.enter_context(tc.tile_pool(name="hp", bufs=2))
        spool = es.enter_context(tc.tile_pool(name="sp", bufs=2))
        tpool = es.enter_context(tc.tile_pool(name="tp", bufs=2))
        ps_bb = es.enter_context(tc.tile_pool(name="psbb", bufs=1, space="PSUM"))
        ps_hz = es.enter_context(tc.tile_pool(name="pshz", bufs=2, space="PSUM"))
        ps_mu = es.enter_context(tc.tile_pool(name="psmu", bufs=2, space="PSUM"))
        ps_ms = es.enter_context(tc.tile_pool(name="psms", bufs=3, space="PSUM"))

        # --- load constants once ---
        wt = {}
        for n, shp in {**wnames_r, **wnames_f}.items():
            dt = F32R if n in wnames_r else F32
            if n == "lsC":
                # rhs for the pi_r sums matmul lives at partitions 64:128,
                # and matmul requires lhsT/rhs base partitions to match.
                t128 = consts.tile([128, shp[1]], dt, tag=n)
                nc.sync.dma_start(out=t128[64:128, :], in_=dram_w[n][:, :])
                wt[n] = t128[64:128, :]
                continue
            t = consts.tile(shp, dt, tag=n)
            nc.sync.dma_start(out=t, in_=dram_w[n][:, :])
            wt[n] = t

        # DRAM output views indexed [st] -> [h, g, k, n]
        def view_full(o, b_):
            return o[:, b_, :, :].rearrange("h k (s g n) -> s h g k n",
                                            s=ST_PER_B, g=G, n=COLS)

        def view_full1(o, b_):
            return o[0, b_, :, :].rearrange("k (s g n) -> s g k n",
                                            s=ST_PER_B, g=G, n=COLS)

        for b_ in range(B):
            vA = view_full(oA, b_)
            vE = view_full1(oE, b_)
            vB = view_full(oB, b_)
            vC = view_full(oC, b_)
            vD = view_full(oD, b_)
            for st in range(ST_PER_B):
                base = st * G * COLS
                x_t = xpool.tile([24, COLS], F32R, tag="x")
                nc.sync.dma_start(
                    out=x_t,
                    in_=xin[b_, :, base:base + G * COLS].rearrange(
                        "c (g n) -> g c n", n=COLS),
                )

                # --- backbone ---
                h1 = hpool.tile([128, COLS], F32R, tag="h1")
                for q in range(NQ):
                    qs = slice(q * 512, (q + 1) * 512)
                    ps = ps_bb.tile([128, 512], F32, tag="bb")
                    nc.tensor.matmul(ps, wt["lw1"], x_t[:, qs],
                                     start=True, stop=True)
                    nc.vector.tensor_scalar(h1[:, qs], ps, 0.0, None, ALU.max)
                h2 = hpool.tile([128, COLS], F32R, tag="h2")
                for q in range(NQ):
                    qs = slice(q * 512, (q + 1) * 512)
                    ps = ps_bb.tile([128, 512], F32, tag="bb")
                    nc.tensor.matmul(ps, wt["lw2"], h1[:, qs],
                                     start=True, stop=True)
                    nc.vector.tensor_scalar(h2[:, qs], ps, wt["bb2"], 0.0,
                                            ALU.add, ALU.max)
                lat = hpool.tile([128, COLS], F32R, tag="lat")
                for q in range(NQ):
                    qs = slice(q * 512, (q + 1) * 512)
                    ps = ps_bb.tile([128, 512], F32, tag="bb")
                    nc.tensor.matmul(ps, wt["lw3"], h2[:, qs],
                                     start=True, stop=True)
                    nc.vector.tensor_scalar(lat[:, qs], ps, wt["bb3"], 0.0,
                                            ALU.add, ALU.max)

                # --- mu chunks (A: mu_r|mu_g, E: mu_b) ---
                sA = spool.tile([128, COLS], F32, tag="sA")
                sE = spool.tile([64, COLS], F32, tag="sE")
                for q in range(NQ):
                    qs = slice(q * 512, (q + 1) * 512)
                    psA = ps_mu.tile([128, 512], F32, tag="mu")
                    nc.tensor.matmul(psA, wt["lA"], lat[:, qs],
                                     start=True, stop=False)
                    nc.tensor.matmul(psA, wt["lrA"], x_t[:, qs],
                                     start=False, stop=True)
                    nc.scalar.copy(sA[:, qs], psA)
                    psE = ps_mu.tile([64, 512], F32, tag="mu")
                    nc.tensor.matmul(psE, wt["lE"], lat[:, qs],
                                     start=True, stop=False)
                    nc.tensor.matmul(psE, wt["lrE"], x_t[:, qs],
                                     start=False, stop=True)
                    nc.vector.tensor_copy(sE[:, qs], psE)
                nc.sync.dma_start(out=vA[st, 0], in_=sA[0:64, :])
                nc.sync.dma_start(out=vA[st, 1], in_=sA[64:128, :])
                nc.sync.dma_start(out=vE[st], in_=sE)

                # --- sg/pi z + exp ---
                sB = spool.tile([128, COLS], F32, tag="sB")
                sC = spool.tile([128, COLS], F32R, tag="sC")
                sD = spool.tile([128, COLS], F32R, tag="sD")
                for name, stile, btile in (("lB", sB, "bB"), ("lC", sC, "bC"),
                                           ("lD", sD, "bD")):
                    for q in range(NQ):
                        qs = slice(q * 512, (q + 1) * 512)
                        psz = ps_hz.tile([128, 512], F32, tag="hz")
                        nc.tensor.matmul(psz, wt[name], lat[:, qs],
                                         start=True, stop=True)
                        nc.scalar.activation(stile[:, qs], psz, AF.Exp,
                                             bias=wt[btile])

                # softplus finalize: ln(e + 1)
                nc.scalar.activation(sB, sB, AF.Ln, bias=1.0)
                nc.scalar.activation(sC[0:64, :], sC[0:64, :], AF.Ln, bias=1.0)
                nc.sync.dma_start(out=vB[st, 0], in_=sB[0:64, :])
                nc.sync.dma_start(out=vB[st, 1], in_=sB[64:128, :])

                # --- softmax normalize pi (C[64:] = pi_r, D = pi_g|pi_b) ---
                for q in range(NQ):
                    qs = slice(q * 512, (q + 1) * 512)
                    ss = ps_ms.tile([12, 512], F32, tag="ms")
                    nc.tensor.matmul(ss, wt["lsC"], sC[64:128, qs],
                                     start=True, stop=False)
                    nc.tensor.matmul(ss, wt["lsD"], sD[:, qs],
                                     start=False, stop=True)
                    tl = tpool.tile([12, 512], F32, tag="tl")
                    nc.scalar.activation(tl, ss, AF.Ln)
                    rs = tpool.tile([12, 512], F32R, tag="rs")
                    nc.scalar.activation(rs, tl, AF.Exp, scale=-1.0)
                    bcC = ps_ms.tile([64, 512], F32, tag="ms")
                    nc.tensor.matmul(bcC, wt["lbC"], rs, start=True, stop=True)
                    nc.vector.tensor_tensor(sC[64:128, qs], sC[64:128, qs],
                                            bcC, ALU.mult)
                    bcD = ps_ms.tile([128, 512], F32, tag="ms")
                    nc.tensor.matmul(bcD, wt["lbD"], rs, start=True, stop=True)
                    nc.vector.tensor_tensor(sD[:, qs], sD[:, qs], bcD,
                                            ALU.mult)
                nc.sync.dma_start(out=vC[st, 0], in_=sC[0:64, :])
                nc.sync.dma_start(out=vC[st, 1], in_=sC[64:128, :])
                nc.sync.dma_start(out=vD[st, 0], in_=sD[0:64, :])
                nc.sync.dma_start(out=vD[st, 1], in_=sD[64:128, :])

    nc.compile()
    return nc


def _head_block(w):
    # lhsT block for one head: [32, 16] with lhsT[j, k] = w[k, j]
    return np.ascontiguousarray(w.T)


def _prep_weights(i):
    f = np.float32
    lw1 = np.zeros((24, 128), f)
    for g in range(G):
        lw1[6 * g:6 * g + 5, 32 * g:32 * (g + 1)] = i["w1"].T
        lw1[6 * g + 5, 32 * g:32 * (g + 1)] = i["b1"]
    lw2 = np.zeros((128, 128), f)
    lw3 = np.zeros((128, 128), f)
    for g in range(G):
        lw2[32 * g:32 * (g + 1), 32 * g:32 * (g + 1)] = i["w2"].T
        lw3[32 * g:32 * (g + 1), 32 * g:32 * (g + 1)] = i["w3"].T

    def head_chunk(w_lo, w_hi):
        l = np.zeros((128, 128 if w_hi is not None else 64), f)
        for g in range(G):
            l[32 * g:32 * (g + 1), 16 * g:16 * (g + 1)] = w_lo.T
            if w_hi is not None:
                l[32 * g:32 * (g + 1), 64 + 16 * g:64 + 16 * (g + 1)] = w_hi.T
        return l

    lA = head_chunk(i["rmu_w"], i["gmu_w"])
    lB = head_chunk(i["rsg_w"], i["gsg_w"])
    lC = head_chunk(i["bsg_w"], i["rpi_w"])
    lD = head_chunk(i["gpi_w"], i["bpi_w"])
    lE = head_chunk(i["bmu_w"], None)

    lrA = np.zeros((24, 128), f)
    lrE = np.zeros((24, 64), f)
    for g in range(G):
        for k in range(K):
            lrA[6 * g + 0, 16 * g + k] = 1.0          # + x_r for mu_r
            lrA[6 * g + 5, 16 * g + k] = i["rmu_b"][k]
            lrA[6 * g + 1, 64 + 16 * g + k] = 1.0     # + x_g for mu_g
            lrA[6 * g + 5, 64 + 16 * g + k] = i["gmu_b"][k]
            lrE[6 * g + 2, 16 * g + k] = 1.0          # + x_b for mu_b
            lrE[6 * g + 5, 16 * g + k] = i["bmu_b"][k]

    lsC = np.zeros((64, 12), f)
    lsD = np.zeros((128, 12), f)
    lbC = np.zeros((12, 64), f)
    lbD = np.zeros((12, 128), f)
    for g in range(G):
        lsC[16 * g:16 * (g + 1), g] = 1.0             # pi_r sums
        lsD[16 * g:16 * (g + 1), 4 + g] = 1.0         # pi_g sums
        lsD[64 + 16 * g:64 + 16 * (g + 1), 8 + g] = 1.0  # pi_b sums
        lbC[g, 16 * g:16 * (g + 1)] = 1.0
        lbD[4 + g, 16 * g:16 * (g + 1)] = 1.0
        lbD[8 + g, 64 + 16 * g:64 + 16 * (g + 1)] = 1.0

    col = lambda v: np.ascontiguousarray(v.reshape(-1, 1).astype(f))
    bb2 = col(np.tile(i["b2"], G))
    bb3 = col(np.tile(i["b3"], G))
    bB = col(np.concatenate([np.tile(i["rsg_b"], G), np.tile(i["gsg_b"], G)]))
    bC = col(np.concatenate([np.tile(i["bsg_b"], G), np.tile(i["rpi_b"], G)]))
    bD = col(np.concatenate([np.tile(i["gpi_b"], G), np.tile(i["bpi_b"], G)]))

    return {"lw1": lw1, "lw2": lw2, "lw3": lw3, "lA": lA, "lB": lB, "lC": lC,
            "lD": lD, "lE": lE, "lrA": lrA, "lrE": lrE, "lsC": lsC,
            "lsD": lsD, "lbC": lbC, "lbD": lbD, "bb2": bb2, "bb3": bb3,
            "bB": bB, "bC": bC, "bD": bD}


def _get_runner():
    """Compile the Bass program once and wrap it in a cached sharded jit."""
    if "runner" in _CACHE:
        return _CACHE["runner"]
    import jax
    from jax.sharding import Mesh, PartitionSpec
    from jax.experimental.shard_map import shard_map
    import concourse.mybir as mb
    import concourse.bass2jax as b2j

    nc = _CACHE.get("nc")
    if nc is None:
        nc = _CACHE["nc"] = _build_program()

    b2j.install_neuronx_cc_hook()
    partition_name = (nc.partition_id_tensor.name
                      if nc.partition_id_tensor else None)
    in_names, out_names, out_avals = [], [], []
    for alloc in nc.m.functions[0].allocations:
        if not isinstance(alloc, mb.MemoryLocationSet):
            continue
        name = alloc.memorylocations[0].name
        if alloc.kind == "ExternalInput":
            if name != partition_name:
                in_names.append(name)
        elif alloc.kind == "ExternalOutput":
            out_names.append(name)
            out_avals.append(jax.core.ShapedArray(
                tuple(alloc.tensor_shape), mb.dt.np(alloc.dtype)))
    n_params = len(in_names)
    bind_names = list(in_names + out_names)
    if partition_name is not None:
        bind_names.append(partition_name)
    bind_names = tuple(bind_names)

    def _body(*args):
        operands = list(args)
        if partition_name is not None:
            operands.append(b2j.partition_id_tensor())
        outs = b2j._bass_exec_p.bind(
            *operands,
            out_avals=tuple(out_avals),
            in_names=bind_names,
            out_names=tuple(out_names),
            lowering_input_output_aliases=(),
            sim_require_finite=True,
            sim_require_nnan=True,
            nc=nc,
        )
        return tuple(outs)

    devices = jax.devices()[:NCORES]
    mesh = Mesh(np.asarray(devices), ("core",))
    nin = n_params + len(out_names)
    fn = jax.jit(
        shard_map(_body, mesh=mesh,
                  in_specs=(PartitionSpec("core"),) * nin,
                  out_specs=(PartitionSpec("core"),) * len(out_names),
                  check_rep=False),
        keep_unused=True,
    )
    zeros = [np.zeros((NCORES * a.shape[0], *a.shape[1:]), a.dtype)
             for a in out_avals]
    runner = {"fn": fn, "in_names": in_names, "out_names": out_names,
              "out_avals": out_avals, "zeros": zeros, "mesh": mesh}
    _CACHE["runner"] = runner
    return runner


def _make_concat_inputs(inputs):
    wmaps = _prep_weights(inputs)
    x = inputs["x"]  # [B, 5, H, W]
    xs = []
    for c in range(NCORES):
        xc = x[:, :, c * HC:(c + 1) * HC, :].reshape(B, CIN, PXB)
        xa = np.empty((B, CIN + 1, PXB), np.float32)
        xa[:, :CIN] = xc
        xa[:, CIN] = 1.0
        xs.append(xa)
    per_core = {"xin": np.concatenate(xs, axis=0)}
    for n, w in wmaps.items():
        per_core[n] = np.concatenate([w] * NCORES, axis=0)
    return per_core


def kernel(**inputs):
    inputs = {k: np.asarray(v, dtype=np.float32) for k, v in inputs.items()}
    runner = _get_runner()
    concat = _make_concat_inputs(inputs)
    args = [concat[n] for n in runner["in_names"]]
    outs = runner["fn"](*args, *runner["zeros"])
    res = {}
    for name, aval, arr in zip(runner["out_names"], runner["out_avals"], outs):
        res[name] = np.asarray(arr).reshape(NCORES, *aval.shape)

    def gather(name, h_idx):
        parts = [res[name][c][h_idx].reshape(B, K, HC, W)
                 for c in range(NCORES)]
        return np.concatenate(parts, axis=2)

    mu_r, mu_g = gather("oA", 0), gather("oA", 1)
    mu_b = gather("oE", 0)
    sg_r, sg_g = gather("oB", 0), gather("oB", 1)
    sg_b, pi_r = gather("oC", 0), gather("oC", 1)
    pi_g, pi_b = gather("oD", 0), gather("oD", 1)
    return (mu_r, sg_r, pi_r, mu_g, sg_g, pi_g, mu_b, sg_b, pi_b)
